# revision 1
# baseline (speedup 1.0000x reference)
"""Trainium2 Bass kernel for nn_CollatedVanillaCNN.

The model applies a tiny CNN (log1p -> conv3x3(16->32)+bn+relu+avgpool2 ->
conv3x3(32->64)+bn+relu+avgpool2 -> fc(64->16)+bn+relu -> fc(16->8) -> expm1)
independently to the 4x4 sliding window at every pixel of x[4,16,128,128]
(zero-padded right/bottom), producing out[4,8,128,128].

Strategy: every output pixel is an independent sample => express the whole
network as 4 dense matmul stages over pixels (features on SBUF partitions,
pixels on the free dim):

  conv1 : windows  K=(sh,sw,c)=256  ->  M=(pw,qw,o1)=512   (masked 3x3 taps)
  conv2 : K=(pw,qw,o1)=512 -> M=(r,t,o2)=256               (avgpool1 folded in)
  fc1   : K=(r,t,o2)=256   -> M=16                         (avgpool2 folded in)
  fc2   : K=16 -> M=8

bn scales are folded into the weight columns; bn/conv biases are applied via
per-partition bias operands of the scalar-engine activation (relu / exp) or
vector-engine tensor_scalar.  Matmuls run as float32r (full-rate fp32).

Sharding: pure data parallel over B x H/2: core = (b, row half), 8192 pixels
per core, tiled as 16 tiles of 512 pixels (4 image rows).  Host does only
data movement (pad/im2col/layout); all arithmetic runs on device.
"""

import ml_dtypes
import numpy as np

import concourse.bacc as bacc
import concourse.bass as bass
import concourse.mybir as mybir
import concourse.tile as tile
from concourse import bass_utils

AF = mybir.ActivationFunctionType
ALU = mybir.AluOpType
F32 = mybir.dt.float32
F32R = mybir.dt.float32r
BF16 = mybir.dt.bfloat16

EPS = 1e-5
NCORES = 8
NT = 16          # pixel tiles per core (each 4 image rows x 128 cols = 512 px)


# ---------------------------------------------------------------- host packing

def _pack_weights(p):
    """Pack all network params into device-layout matmul weights / biases."""
    w1 = p["conv1_w"].astype(np.float64)   # [32,16,3,3]
    w2 = p["conv2_w"].astype(np.float64)   # [64,32,3,3]
    s1 = (p["bn1_g"] / np.sqrt(p["bn1_v"] + EPS)).astype(np.float64)
    s2 = (p["bn2_g"] / np.sqrt(p["bn2_v"] + EPS)).astype(np.float64)
    s3 = (p["bn3_g"] / np.sqrt(p["bn3_v"] + EPS)).astype(np.float64)

    # conv1: rows (h,p) -> (sh,c,sw), cols f=(pw*4+qw)*32+o
    W1 = np.zeros((2, 128, 512), np.float64)
    pp = np.arange(128)
    c_of_p = (pp % 64) // 4
    sw_of_p = pp % 4
    f = np.arange(512)
    pw_of_f = f // 128
    qw_of_f = (f % 128) // 32
    o_of_f = f % 32
    for h in range(2):
        sh = 2 * h + pp // 64                       # [128]
        du = sh[:, None] - pw_of_f[None, :] + 1      # [128,512]
        dv = sw_of_p[:, None] - qw_of_f[None, :] + 1
        valid = (du >= 0) & (du < 3) & (dv >= 0) & (dv < 3)
        duc = np.clip(du, 0, 2)
        dvc = np.clip(dv, 0, 2)
        vals = w1[o_of_f[None, :].repeat(128, 0),
                  c_of_p[:, None].repeat(512, 1),
                  duc, dvc]
        W1[h] = np.where(valid, vals, 0.0) * s1[o_of_f][None, :]
    W1 = W1.transpose(1, 0, 2).reshape(128, 1024)    # [k, h*512+f]
    bias1 = ((p["conv1_b"] - p["bn1_m"]) * s1 + p["bn1_b"])  # [32] by o
    b1 = np.tile(bias1, 4).reshape(128, 1)           # partition (qw*32+o)

    # conv2 (+pool1 qw-fold, 1/4): rows (qw*32+o1),
    # cols ff = r*256 + r2*128 + t2*64 + o2 (pw pre-folded on DVE).
    kk = np.arange(128)
    o1_k = kk % 32
    t_k = (kk // 32) // 2
    ff = np.arange(512)
    r_f = ff // 256
    r2_f = (ff % 256) // 128
    t2_f = (ff % 128) // 64
    o2_f = ff % 64
    W2 = 0.25 * w2[o2_f[None, :].repeat(128, 0),
                   o1_k[:, None].repeat(512, 1),
                   (r_f - r2_f + 1)[None, :].repeat(128, 0),
                   t_k[:, None] - t2_f[None, :] + 1] * s2[o2_f][None, :]
    bias2 = ((p["conv2_b"] - p["bn2_m"]) * s2 + p["bn2_b"])  # [64] by o2
    b2 = np.tile(bias2, 2).reshape(128, 1)           # partition (t2*64+o2)

    # fc1 (+avgpool2): chunk r, rows (t*64+o2).  Eight slot variants per
    # chunk: variant k writes only output partitions 16k..16k+16 (other
    # cols zero); 16 accumulating matmuls pack 8 tiles into one PSUM bank.
    base = 0.25 * p["fc1_w"].astype(np.float64).T * s3[None, :]  # [64,16]
    w3c = np.stack([np.tile(base, (2, 1))] * 2)      # [2,128,16] per chunk r
    W3 = np.zeros((2, 128, 8 * 128), np.float64)
    for k8 in range(8):
        W3[:, :, 128 * k8 + 16 * k8:128 * k8 + 16 * k8 + 16] = w3c
    W3 = W3.transpose(1, 0, 2).reshape(128, 2048)    # [row, r*1024 + slotcol]
    b3v = (p["fc1_b"] - p["bn3_m"]) * s3 + p["bn3_b"]
    b3 = np.tile(b3v, 8).reshape(128, 1)             # partition (k,m)

    # fc2 (fp32), block-diag over the 8 slots; two group variants: group g
    # writes cols 64g..64g+64 (= (k,o)), accumulated into one bank.
    W4 = np.zeros((128, 2 * 128), np.float64)
    for g in range(2):
        for k8 in range(8):
            W4[16 * k8:16 * k8 + 16,
               128 * g + 64 * g + 8 * k8:128 * g + 64 * g + 8 * k8 + 8] = \
                p["fc2_w"].astype(np.float64).T
    b4 = np.tile(p["fc2_b"], 16).reshape(128, 1)     # partition (g,k,o)

    # Combined device layouts: one weight tensor (f32r) + one fp32 tensor.
    wtot = np.zeros((128, 1024 + 512 + 2048), np.float64)
    wtot[:, 0:1024] = W1
    wtot[:, 1024:1536] = W2
    wtot[:, 1536:3584] = W3
    btot = np.zeros((128, 4 + 256), np.float64)
    btot[:, 0:1] = b1
    btot[:, 1:2] = b2
    btot[:, 2:3] = b3
    btot[:, 3:4] = b4
    btot[:, 4:260] = W4
    return {
        "wtot": np.ascontiguousarray(wtot, np.float32),
        "btot": np.ascontiguousarray(btot, np.float32),
    }


def _im2col_core(xs):
    """xs: [16, 67, 131] padded row-slab -> xcols [16 tiles, 128, 1024]."""
    xcols = np.empty((NT, 128, 1024), np.float32)
    for sh in range(4):
        h, lo = divmod(sh, 2)
        for sw in range(4):
            blk = xs[:, sh:sh + 64, sw:sw + 128]         # [16c, 64, 128]
            blk = blk.reshape(16, NT, 4 * 128)           # [c, t, px]
            parts = 64 * lo + np.arange(16) * 4 + sw     # dest partitions
            xcols[:, parts, 512 * h:512 * h + 512] = blk.transpose(1, 0, 2)
    return xcols


def _make_in_maps(inputs):
    x = np.asarray(inputs["x"], np.float32)              # [4,16,128,128]
    xp = np.pad(x, ((0, 0), (0, 0), (0, 3), (0, 3)))     # [4,16,131,131]
    packed = _pack_weights({k: np.asarray(v, np.float64) for k, v in inputs.items()
                            if k != "x"})
    in_maps = []
    for core in range(NCORES):
        b, half = divmod(core, 2)
        r0 = half * 64
        xs = xp[b, :, r0:r0 + 67, :]
        m = dict(packed)
        m["xcols"] = _im2col_core(xs)
        in_maps.append(m)
    return in_maps


# ---------------------------------------------------------------- device build

def _pin_act_table_set():
    """Force every activation onto natural_log_exp_and_others (has Ln, Exp,
    Relu, Copy): the default per-function greedy set choice alternates table
    sets across Ln/Relu/Exp and burns ~2.7us per ACT_TABLE_LOAD, 28x."""
    from concourse.hw_specs import get_activation_tables as orig
    keep = "natural_log_exp_and_others"

    def patched(arch):
        t = orig(arch)
        return {name: (funcs if name == keep else set())
                for name, funcs in t.items()}

    bacc.get_activation_tables = patched


def build_nc():
    _pin_act_table_set()
    nc = bacc.Bacc("TRN2", target_bir_lowering=False, debug=False,
                   num_devices=NCORES)
    xcols_d = nc.dram_tensor("xcols", [NT, 128, 1024], F32, kind="ExternalInput")
    wtot_d = nc.dram_tensor("wtot", [128, 3584], F32R, kind="ExternalInput")
    btot_d = nc.dram_tensor("btot", [128, 260], F32, kind="ExternalInput")
    out_d = nc.dram_tensor("out", [8, 64, 128], F32, kind="ExternalOutput")

    with tile.TileContext(nc) as tc:
        with (
            tc.tile_pool(name="wpool", bufs=1) as wpool,
            tc.tile_pool(name="xin", bufs=3) as xin,
            tc.tile_pool(name="rhs1", bufs=2) as rhs1p,
            tc.tile_pool(name="relu1", bufs=2) as relu1p,
            tc.tile_pool(name="relu2", bufs=2) as relu2p,
            tc.tile_pool(name="arpool", bufs=2) as arp,
            tc.tile_pool(name="relu3", bufs=2) as relu3p,
            tc.tile_pool(name="outsb", bufs=2) as outp,
            tc.tile_pool(name="ps1", bufs=2, space="PSUM") as ps1,
            tc.tile_pool(name="ps2", bufs=1, space="PSUM") as ps2,
            tc.tile_pool(name="psf1", bufs=1, space="PSUM") as psf1,
            tc.tile_pool(name="psf2", bufs=1, space="PSUM") as psf2,
        ):
            ws = wpool.tile([128, 3584], F32R, tag="ws")
            bs = wpool.tile([128, 260], F32, tag="bs")
            w1s = ws[:, 0:1024]
            w2s = ws[:, 1024:1536]
            w3s = ws[:, 1536:3584]
            b1s = bs[:, 0:1]
            b2s = bs[:, 1:2]
            b3s = bs[:, 2:3]
            b4s = bs[:, 3:4]
            w4s = bs[:, 4:260]

            # Software-pipelined over tiles with a 4-stage skew so every
            # engine always has only cross-step dependencies in its stream:
            #   F(t0): load, log1p, conv1
            #   A(t1): bn1 (scalar engine, reads last step's PSUM)
            #   M(t2): conv2, bn2
            #   B(t3): fc1, bn3, fc2, exp, -1, store
            xts = {}
            r1s = {}
            o1s = {}
            rl1s = {}
            rl2s = {}

            def dma_in(t):
                xts[t] = xin.tile([128, 1024], F32, name="xt", tag="xt")
                nc.sync.dma_start(xts[t][:], xcols_d[t])

            def warmup():
                # Keep the PE busy during the input-DMA prologue so the HAM
                # clock gate reaches 8/8 before the first real matmul.
                dmy = wpool.tile([128, 8], F32, tag="dmy")
                dmyp = ps1.tile([128, 1024], F32, tag="o1")
                nc.vector.memset(dmy[:], 0.0)
                for _ in range(48):
                    nc.tensor.matmul(dmyp[0:8, 0:8], dmy[:], dmy[:],
                                     start=True, stop=True)

            def front_ln(t):
                r1s[t] = rhs1p.tile([128, 1024], F32R, name="r1", tag="r1")
                nc.scalar.activation(r1s[t][:], xts[t][:], AF.Ln,
                                     bias=1.0, scale=1.0)
                del xts[t]

            def front_conv1(t):
                bn1_inline = True
                # conv1 (+bn1 scale): K=256, M=512.  Banded in (sh vs pw):
                # output chunk pw only needs window rows sh in
                # {pw-1,pw,pw+1} & [0,3], so 6 matmuls instead of 8.
                # rhs1 chunk h holds sh=2h (parts 0:64) / sh=2h+1 (64:128).
                r1 = r1s[t]
                rl1s[t] = relu1p.tile([128, 2048], F32R, name="rl1", tag="rl1")
                for half in range(2):           # halves: m in {0,1} / {2,3}
                    o1 = ps1.tile([128, 1024], F32, tag="o1")
                    if half == 0:
                        # m=0: sh{0,1} = chunk0 only
                        nc.tensor.matmul(o1[:, 0:512], w1s[:, 0:128],
                                         r1[:, 0:512], start=True, stop=True)
                        # m=1: sh{0,1,2}; chunk1 rows sh=3 are zero weights
                        nc.tensor.matmul(o1[:, 512:1024], w1s[:, 128:256],
                                         r1[:, 0:512], start=True, stop=False)
                        nc.tensor.matmul(o1[:, 512:1024],
                                         w1s[:, 512 + 128:512 + 256],
                                         r1[:, 512:1024],
                                         start=False, stop=True)
                    else:
                        # m=2: sh{1,2,3}; chunk0 rows sh=0 are zero weights
                        nc.tensor.matmul(o1[:, 0:512],
                                         w1s[:, 256:384],
                                         r1[:, 0:512],
                                         start=True, stop=False)
                        nc.tensor.matmul(o1[:, 0:512],
                                         w1s[:, 512 + 256:512 + 384],
                                         r1[:, 512:1024], start=False,
                                         stop=True)
                        # m=3: sh{2,3} = chunk1 only
                        nc.tensor.matmul(o1[:, 512:1024],
                                         w1s[:, 512 + 384:512 + 512],
                                         r1[:, 512:1024], start=True,
                                         stop=True)
                    # bn1 bias + relu on this half
                    nc.scalar.activation(
                        rl1s[t][:, 1024 * half:1024 * (half + 1)],
                        o1[:], AF.Relu, bias=b1s[:], scale=1.0)
                del r1s[t]

            ars = {}

            def pwadd(t, r):
                # pool1 pw-fold: A_r = rl1[:, h=r, pwlo=0] + rl1[:, h=r, 1]
                if r == 0:
                    ars[t] = arp.tile([128, 1024], F32R, name="ar", tag="ar")
                rl1 = rl1s[t]
                nc.vector.tensor_tensor(
                    ars[t][:, 512 * r:512 * r + 512],
                    rl1[:, 1024 * r:1024 * r + 512],
                    rl1[:, 1024 * r + 512:1024 * r + 1024], op=ALU.add)
                if r == 1:
                    del rl1s[t]

            def mid_half(t, n):
                # conv2 (+pool1 qw-fold, +bn2 scale): M-chunk n (=r2),
                # 2 accumulating matmuls K=(qw,o1)=128 over r + DVE relu.
                ar = ars[t]
                if n == 0:
                    rl2s[t] = relu2p.tile([128, 1024], F32R, name="rl2",
                                          tag="rl2")
                rl2 = rl2s[t]
                o2 = ps2.tile([128, 512], F32, tag="o2")
                for r in range(2):
                    nc.tensor.matmul(
                        o2[:],
                        w2s[:, 256 * r + 128 * n:256 * r + 128 * n + 128],
                        ar[:, 512 * r:512 * r + 512],
                        start=(r == 0), stop=(r == 1),
                    )
                nc.vector.tensor_scalar(rl2[:, 512 * n:512 * (n + 1)],
                                        o2[:], b2s[:], 0.0,
                                        op0=ALU.add, op1=ALU.max)
                if n == 1:
                    del ars[t]

            f1banks = {}
            f2bank = psf2.tile([128, 512], F32, name="f2bank")

            def back_fc1(t):
                # fc1 (+pool2, +bn3 scale): K=256 (2 chunks), M=128 with
                # slot-k-selective weight cols; 8 tiles accumulation-packed
                # into one PSUM bank.
                rl2 = rl2s[t]
                k8 = t % 8
                if k8 == 0:
                    f1banks[t // 8] = psf1.tile([128, 512], F32, name="f1",
                                                tag="f1")
                f1 = f1banks[t // 8]
                for h in range(2):
                    nc.tensor.matmul(
                        f1[:],
                        w3s[:, 1024 * h + 128 * k8:1024 * h + 128 * k8 + 128],
                        rl2[:, 512 * h:512 * h + 512],
                        start=(k8 == 0 and h == 0), stop=(k8 == 7 and h == 1),
                    )
                del rl2s[t]

            def back_fc2(g):
                # relu3 on the packed bank, then one fp32 block-diag fc2
                # matmul per 8-tile group, group-packed into f2bank.
                rl3 = relu3p.tile([128, 512], F32, name="rl3", tag="rl3")
                nc.scalar.activation(rl3[:], f1banks[g][:], AF.Relu,
                                     bias=b3s[:], scale=1.0)
                nc.tensor.matmul(f2bank[:], w4s[:, 128 * g:128 * g + 128],
                                 rl3[:], start=(g == 0), stop=(g == 1))
                del f1banks[g]

            def finale():
                # expm1 = exp(x + fc2_b) - 1 over all 16 tiles at once
                ob = outp.tile([128, 512], F32, name="ob", tag="ob")
                nc.scalar.activation(ob[:], f2bank[:], AF.Exp,
                                     bias=b4s[:], scale=1.0)
                ob2 = outp.tile([128, 512], F32, name="ob2", tag="ob2")
                nc.vector.tensor_scalar(ob2[:], ob[:], 1.0,
                                        None, op0=ALU.subtract)
                dst = out_d[:].copy()
                dst.ap = mybir.VecI64Pair(
                    [(4096, 2), (512, 8), (8192, 8), (128, 4), (1, 128)])
                dst.offset = 0
                nc.sync.dma_start(dst, ob2[:])

            dma_in(0)
            nc.scalar.dma_start(ws[:], wtot_d[:])
            nc.scalar.dma_start(bs[:], btot_d[:])
            warmup()
            for s in range(NT + 2):
                t0, t1, t2 = s, s - 1, s - 2
                if s + 1 < NT:
                    dma_in(s + 1)
                if t0 < NT:
                    front_ln(t0)
                if 0 <= t1 < NT:
                    mid_half(t1, 0)
                if 0 <= t2 < NT:
                    back_fc1(t2)
                if 0 <= t1 < NT:
                    mid_half(t1, 1)
                if 0 <= t2 < NT:
                    if t2 % 8 == 7:
                        back_fc2(t2 // 8)
                    if t2 == NT - 1:
                        finale()
                if t0 < NT:
                    front_conv1(t0)
                    pwadd(t0, 0)
                    pwadd(t0, 1)

    nc.compile()
    return nc


_NC = None


def _get_nc():
    global _NC
    if _NC is None:
        _NC = build_nc()
    return _NC


def _assemble(results):
    out = np.empty((4, 8, 128, 128), np.float32)
    for core in range(NCORES):
        b, half = divmod(core, 2)
        out[b, :, half * 64:half * 64 + 64, :] = results[core]["out"]
    return out


def kernel(_trace=False, **inputs):
    nc = _get_nc()
    in_maps = _make_in_maps(inputs)
    res = bass_utils.run_bass_kernel_spmd(
        nc, in_maps, core_ids=list(range(NCORES)), trace=_trace)
    out = _assemble(res.results)
    if _trace:
        return out, res
    return out


def kernel_sim(cores=None, **inputs):
    from concourse.bass_interp import CoreSim
    nc = _get_nc()
    in_maps = _make_in_maps(inputs)
    outs = []
    for core in (cores if cores is not None else range(NCORES)):
        sim = CoreSim(nc, trace=False, require_finite=False,
                      require_nnan=False)
        for k, v in in_maps[core].items():
            sim.tensor(k)[:] = v
        sim.simulate()
        outs.append({"out": sim.tensor("out").copy()})
    return outs

